# revision 21
# baseline (speedup 1.0000x reference)
"""Trainium2 Bass kernel for nn_Aggregate (gate-softmax graph pooling).

Computes, for each graph b:
    gate[b,n] = x[b,n,:] @ W1 + b1
    attn      = softmax(gate[b,:])
    y[b,:]    = sum_n attn[b,n] * x[b,n,:]

Strategy (memory-bound; roofline = one HBM read of x):
  - Data-parallel over the 32 graphs: 4 graphs per NeuronCore, 8 cores.
  - Single pass over x. gate values are ~N(0,1) so exp() without the
    max-shift is safe in fp32; softmax = (sum e^g x) / (sum e^g) needs
    no running-max correction, so every x element is read from HBM once.
  - Per 1 MiB slab (1024 nodes as [128 partitions x 8 nodes x 256 feat]):
      DVE : g1 = x * W1rep (one op) + grouped reduce for KDVE node-groups
      ACT : reduce of remaining groups via activation(Copy, accum_out),
            then exp(gates + b1) whose accum_out gives sum(e^g)/partition
      PE  : 8x matmul, stationary = e^g column [128,1], moving = x tile
            [128,256]; accumulates sum_n e^g[n] * x[n,:] into PSUM [1,256]
  - Denominator finishes on host: sum of the per-partition exp-sums.
"""

import numpy as np

import concourse.bass as bass
import concourse.tile as tile
from concourse import mybir
from concourse.bass_utils import run_bass_kernel_spmd

BZ, N, F = 32, 8192, 256
NCORES = 8
BZL = BZ // NCORES  # graphs per core
P = 128             # SBUF partitions
JJ = 8              # nodes per partition per slab
SLAB = P * JJ       # 1024 nodes per slab
FP32 = mybir.dt.float32


def split_multiwait(nc) -> int:
    """Walrus in this image only encodes one sync-wait per instruction for
    ctrl-class ops; hoist extra waits onto single-wait NoOps just before."""
    n_fixed = 0
    for fn in nc.m.functions:
        for blk in fn.blocks:
            new_list = []
            for inst in blk.instructions:
                si = inst.sync_info
                waits = list(si.on_wait) if si is not None else []
                if len(waits) > 1:
                    for k, w in enumerate(waits):
                        new_list.append(
                            mybir.InstNoOp(
                                name=f"{inst.name}-wsplit{k}",
                                engine=inst.engine,
                                sync_info=mybir.SyncInfo(on_wait=[w], on_update=[]),
                                bass_nofuse=True,
                            )
                        )
                    inst.sync_info = mybir.SyncInfo(
                        on_wait=[], on_update=list(si.on_update)
                    )
                    n_fixed += 1
                new_list.append(inst)
            blk.instructions = new_list
    return n_fixed


def build(n_nodes: int = N, bzl: int = BZL, fixup: bool = True) -> bass.Bass:
    nslab = n_nodes // SLAB
    assert nslab * SLAB == n_nodes

    nc = bass.Bass("TRN2", target_bir_lowering=False, debug=False)
    x_d = nc.dram_tensor("x", [bzl, n_nodes, F], FP32, kind="ExternalInput").ap()
    w1_d = nc.dram_tensor("W1", [F, 1], FP32, kind="ExternalInput").ap()
    b1_d = nc.dram_tensor("b1", [1], FP32, kind="ExternalInput").ap()
    y_d = nc.dram_tensor("y_unnorm", [bzl, F], FP32, kind="ExternalOutput").ap()
    ws_d = nc.dram_tensor("wsum", [bzl, P, nslab + 2], FP32, kind="ExternalOutput").ap()

    with tile.TileContext(nc) as tc:
        with (
            tc.tile_pool(name="singles", bufs=1) as singles,
            tc.tile_pool(name="xp", bufs=6) as xp,
            tc.tile_pool(name="g1p", bufs=4) as g1p,
            tc.tile_pool(name="small", bufs=4) as small,
            tc.tile_pool(name="scr", bufs=2) as scrp,
            tc.tile_pool(name="wsump", bufs=2) as wsump,
            tc.tile_pool(name="outp", bufs=2) as outp,
            tc.tile_pool(name="psum", bufs=2, space="PSUM") as psump,
        ):
            # W1 (256 contiguous fp32) broadcast to [128, 256]; the mul reads
            # it through a stride-0 view so no 1 MiB replication is needed.
            # Broadcasts go on the gpsimd DMA queue so the first x-slab load
            # on the sync queue isn't stuck behind them.
            w1rep = singles.tile([P, F], FP32)
            nc.gpsimd.dma_start(
                out=w1rep,
                in_=bass.AP(tensor=w1_d.tensor, offset=w1_d.offset, ap=[[0, P], [1, F]]),
            )
            w1r_ap = w1rep[:, :]
            w1rep_bc = bass.AP(
                tensor=w1r_ap.tensor,
                offset=w1r_ap.offset,
                ap=[list(w1r_ap.ap[0]), [0, JJ], list(w1r_ap.ap[1])],
            )
            # Materialized [128, 8, 256] copy: unit-stride in1 keeps the big
            # DVE multiplies on the flat-2D fast path (stride-0 views cost
            # ~+400ns/op there). Priming chunks use the view; they're tiny.
            w1rep8 = singles.tile([P, JJ, F], FP32)
            nc.gpsimd.tensor_copy(
                w1rep8.rearrange("p j f -> p (j f)"),
                bass.AP(
                    tensor=w1r_ap.tensor,
                    offset=w1r_ap.offset,
                    ap=[list(w1r_ap.ap[0]), [0, JJ], list(w1r_ap.ap[1])],
                ),
            )
            # b1 scalar broadcast to [128,1] for the activation bias.
            b1b = singles.tile([P, 1], FP32)
            nc.gpsimd.dma_start(
                out=b1b,
                in_=bass.AP(tensor=b1_d.tensor, offset=b1_d.offset, ap=[[0, P], [1, 1]]),
            )

            for b in range(bzl):
                wsum_cols = wsump.tile([P, nslab + 2], FP32)
                nc.vector.memset(wsum_cols[:, nslab : nslab + 2], 0.0)
                psum_row = psump.tile([1, F], FP32)

                # Work items: (node_start, jj). The very first slab of the
                # whole kernel is split fine-grained so the PE pipeline
                # primes as early as possible.
                chunks = []
                for s in range(nslab):
                    if b == 0 and s == 0:
                        chunks += [(0, 2), (P * 2, 2), (P * 4, JJ - 4)]
                    else:
                        chunks.append((s * SLAB, JJ))

                for ci, (n0, jj) in enumerate(chunks):
                    # node(p, j) = n0 + p*jj + j: each partition reads
                    # jj KiB contiguous -> fully linear HBM->SBUF DMA.
                    x_sb = xp.tile([P, JJ, F], FP32, tag="x_sb")
                    nc.sync.dma_start(
                        out=x_sb[:, 0:jj, :],
                        in_=x_d[b, n0 : n0 + P * jj, :].rearrange(
                            "(p j) f -> p j f", p=P
                        ),
                    )
                    g1 = g1p.tile([P, JJ, F], FP32, tag="g1")
                    # DVE takes the first jdve node-groups of the multiply,
                    # idle GpSimd the rest (they run on different SBUF port
                    # windows than the 1-port reduces). Priming chunks
                    # (jj < JJ) run DVE-only for the shortest latency chain.
                    jdve = jj if jj < JJ else jj - 2
                    nc.vector.tensor_mul(
                        g1[:, 0:jdve, :].rearrange("p j f -> p (j f)"),
                        x_sb[:, 0:jdve, :].rearrange("p j f -> p (j f)"),
                        w1rep8[:, 0:jdve, :].rearrange("p j f -> p (j f)")
                        if jj == JJ
                        else bass.AP(
                            tensor=w1rep_bc.tensor,
                            offset=w1rep_bc.offset,
                            ap=[list(w1rep_bc.ap[0]), [0, jdve], [1, F]],
                        ),
                    )
                    if jdve < jj:
                        nc.gpsimd.tensor_mul(
                            g1[:, jdve:jj, :].rearrange("p j f -> p (j f)"),
                            x_sb[:, jdve:jj, :].rearrange("p j f -> p (j f)"),
                            w1rep8[:, jdve:jj, :].rearrange("p j f -> p (j f)"),
                        )
                    # Grouped 1x reduce: 5 groups on DVE, rest on ACT's
                    # fused accumulator (DVE marginal ~267ns/group, ACT
                    # ~790ns/group flat-heavy).
                    kdve = min(5, jj)
                    gates = small.tile([P, JJ], FP32, tag="gates")
                    nc.vector.reduce_sum(
                        gates[:, 0:kdve], g1[:, 0:kdve, :], axis=mybir.AxisListType.X
                    )
                    for j in range(kdve, jj):
                        scr2 = scrp.tile([P, F], FP32, tag="scr2")
                        nc.scalar.activation(
                            out=scr2,
                            in_=g1[:, j, :],
                            func=mybir.ActivationFunctionType.Copy,
                            bias=0.0,
                            scale=1.0,
                            accum_out=gates[:, j : j + 1],
                        )
                    w_sb = small.tile([P, JJ], FP32, tag="w")
                    nc.scalar.activation(
                        out=w_sb[:, 0:jj],
                        in_=gates[:, 0:jj],
                        func=mybir.ActivationFunctionType.Exp,
                        bias=b1b,
                        scale=1.0,
                        accum_out=wsum_cols[:, ci : ci + 1],
                    )
                    for j in range(jj):
                        nc.tensor.matmul(
                            out=psum_row,
                            lhsT=w_sb[:, j : j + 1],
                            rhs=x_sb[:, j, :],
                            start=(ci == 0 and j == 0),
                            stop=(ci == len(chunks) - 1 and j == jj - 1),
                        )
                yrow = outp.tile([1, F], FP32)
                nc.vector.tensor_copy(yrow, psum_row)
                nc.sync.dma_start(out=y_d[b : b + 1, :], in_=yrow)
                nc.sync.dma_start(out=ws_d[b], in_=wsum_cols)

    if fixup:
        # CoreSim chokes on the inserted NoOps; only needed for the HW compile.
        split_multiwait(nc)
    return nc


def run(x, W1, b1, trace: bool = False, tmpdir: str | None = None):
    """Shard over cores, execute, and return (y, BassKernelResults)."""
    x = np.ascontiguousarray(np.asarray(x, dtype=np.float32))
    W1 = np.ascontiguousarray(np.asarray(W1, dtype=np.float32))
    b1 = np.ascontiguousarray(np.asarray(b1, dtype=np.float32))
    assert x.shape == (BZ, N, F), x.shape

    nc = build()
    in_maps = [
        {"x": np.ascontiguousarray(x[c * BZL : (c + 1) * BZL]), "W1": W1, "b1": b1}
        for c in range(NCORES)
    ]
    res = run_bass_kernel_spmd(
        nc, in_maps, core_ids=list(range(NCORES)), trace=trace, tmpdir=tmpdir
    )
    y_un = np.concatenate([r["y_unnorm"] for r in res.results], axis=0)  # [32, 256]
    ws = np.concatenate([r["wsum"] for r in res.results], axis=0)        # [32, 128, ns]
    denom = ws.reshape(BZ, -1).astype(np.float64).sum(axis=1)
    y = (y_un.astype(np.float64) / denom[:, None]).astype(np.float32)
    return y, res


def kernel(x, W1, b1):
    y, _ = run(x, W1, b1)
    return y
